# revision 47
# baseline (speedup 1.0000x reference)
"""Multi-head attention with RoPE on 8 Trainium2 NeuronCores.

Strategy: tensor-parallel over heads (16 heads / 8 cores = 2 heads per
core). Each core computes its 2 heads' q/k/v projections, RoPE, full
(non-causal) softmax attention, and a partial output projection over its
128-dim slice of the attention output; the host sums the 8 partial
outputs and adds the output bias.

Layout notes (per core, hidden=1024, S=seq, D=64 head dim):
 - x is pre-transposed on the host to xT [1024, B*S] so projection
   matmuls contract over the hidden dim on SBUF partitions with
   contiguous DMA.
 - q/k are produced directly in [e=128, t] (head-major) layout; RoPE is
   folded into duplicated/permuted projection weights combined
   with host-precomputed cos/sin tables C1/C2, so no on-device
   permutation is needed:  q_roped = (q + bq) * C1 + (swap(q) + swap(bq)) * C2
 - scores are computed transposed (S^T [tk, tq]) so exp(S^T) feeds the
   PV matmul directly with no transposes anywhere.  Softmax max-
   subtraction is skipped (scores are O(1) here), and the denominator is
   obtained by appending a ones column to V (M=65 matmul).  The
   reciprocal is broadcast across partitions with a tiny selector
   matmul, and normalization commutes with the (per-token) out-proj.
 - matmul operands are bf16 (full-rate streaming, half the DMA and
   weight-load traffic); all accumulation stays fp32 in PSUM and the
   bias/rope/softmax arithmetic runs in fp32 on the DVE/ACT before the
   bf16 store.

Scheduling (v2):
 - PE warmup matmuls at kernel start flip the HAM clock gate to 2.4 GHz
   while the input DMAs are still in flight.
 - DMA issue order is critical-path-first: xt chunk 0, then the weights
   the first projection chain needs, then everything else.
 - Batch 0's projection is pipelined INTO batch 0's attention: scores
   for kv-block j only need k-chunk j//4, so qk(0,1..3) and v(0,*) are
   emitted inside the first tq chunk's tk loop.
 - The softmax normalization chain of chunk t is deferred into chunk
   t+1 (as the out-proj already was) so its cross-engine latency hides
   under the next chunk's score stream.
"""

import sys

for _p in ("/opt/trn_rl_repo",):
    if _p not in sys.path:
        sys.path.append(_p)

import numpy as np

import concourse.bacc as bacc
import concourse.bass as bass
import concourse.mybir as mybir
import concourse.tile as tile
from concourse.bass_utils import run_bass_kernel_spmd

F32 = mybir.dt.float32
F32R = mybir.dt.float32r
BF16 = mybir.dt.bfloat16
MMDT = BF16
ALU = mybir.AluOpType

HIDDEN = 1024
HEADS = 16
D = 64
ROPE_BASE = 10000.0
NCORES = 8
HPC = HEADS // NCORES  # heads per core = 2
EPC = HPC * D  # out dims per core = 128
B_FULL, S_FULL = 4, 2048

TQ = 512  # query-chunk width
TK = 128  # key-block width

N_WARMUP = 12  # dummy matmuls to flip the HAM clock gate during DMA load


import ml_dtypes


def to_mm(a):
    """Convert an fp32 array to the matmul operand dtype (bf16, RNE)."""
    return np.ascontiguousarray(np.asarray(a, np.float32).astype(ml_dtypes.bfloat16))


def build_nc(B, S):
    """Build the per-core Bass program (SPMD: all cores run this)."""
    nc = bacc.Bacc(None, target_bir_lowering=False)
    T = B * S
    NTQ = S // TQ  # tq chunks per batch
    NTK = S // TK  # tk blocks per batch
    DC = HIDDEN // 128  # contraction chunks

    xt_d = nc.dram_tensor("xt", [HIDDEN, T], MMDT, kind="ExternalInput")
    wq_d = nc.dram_tensor("wq", [HIDDEN, EPC], MMDT, kind="ExternalInput")
    wk_d = nc.dram_tensor("wk", [HIDDEN, EPC], MMDT, kind="ExternalInput")
    psw_d = nc.dram_tensor("psw", [EPC, EPC], MMDT, kind="ExternalInput")
    wv_d = nc.dram_tensor("wv", [HIDDEN, EPC], MMDT, kind="ExternalInput")
    wo_d = nc.dram_tensor("wo", [EPC, HIDDEN], MMDT, kind="ExternalInput")
    bq4_d = nc.dram_tensor("bq4", [EPC, 4], F32, kind="ExternalInput")
    bvb_d = nc.dram_tensor("bvb", [EPC, EPC], F32, kind="ExternalInput")
    c1_d = nc.dram_tensor("c1", [EPC, S], MMDT, kind="ExternalInput")
    c2_d = nc.dram_tensor("c2", [EPC, S], MMDT, kind="ExternalInput")
    sel2_d = nc.dram_tensor("sel2", [128, EPC], MMDT, kind="ExternalInput")
    yt_d = nc.dram_tensor("yt", [HIDDEN, T], F32, kind="ExternalOutput")

    with tile.TileContext(nc) as tc:
        with (
            tc.tile_pool(name="const", bufs=1) as cpool,
            tc.tile_pool(name="xin", bufs=6) as xpool,
            tc.tile_pool(name="qk", bufs=2) as qkpool,
            tc.tile_pool(name="vsb", bufs=2) as vpool,
            tc.tile_pool(name="esb", bufs=12) as epool,
            tc.tile_pool(name="work", bufs=6) as wpool,
            tc.tile_pool(name="yout", bufs=6) as ypool,
            tc.tile_pool(name="psA", bufs=2, space="PSUM") as psA,
            tc.tile_pool(name="psP", bufs=1, space="PSUM") as psP,
            tc.tile_pool(name="psO", bufs=1, space="PSUM") as psO,
            tc.tile_pool(name="psY", bufs=1, space="PSUM") as psY,
        ):
            # ---- PE warmup: no-input matmuls flip HAM to 2.4 GHz while
            # the first DMAs land; results go to a scratch PSUM tile that
            # nothing reads.
            wz = cpool.tile([128, TQ], MMDT, tag="warm_z")
            nc.gpsimd.memset(wz[:], 0.0)
            wp = psY.tile([128, TQ], F32, tag="y")
            for _ in range(N_WARMUP):
                nc.tensor.matmul(wp[:], wz[:, 0:128], wz[:])

            # ---- input DMAs, critical-path first ----
            xt_tiles = {}

            def load_xt(b, t4):
                tlo = b * S + t4 * TQ
                t = xpool.tile([128, DC, TQ], MMDT, tag="xt")
                nc.sync.dma_start(
                    t[:],
                    xt_d[:, tlo : tlo + TQ].rearrange("(dc p) t -> p dc t", p=128),
                )
                xt_tiles[(b, t4)] = t

            def _ldw(dram):
                t = cpool.tile([128, DC, EPC], MMDT, tag=dram.name + "_sb")
                nc.sync.dma_start(t[:], dram[:, :].rearrange("(dc p) m -> p dc m", p=128))
                return t

            load_xt(0, 0)
            wq = _ldw(wq_d)
            wk = _ldw(wk_d)
            c1 = cpool.tile([EPC, S], MMDT, tag="c1_sb")
            nc.sync.dma_start(c1[:], c1_d[:, :])
            bq4 = cpool.tile([EPC, 4], F32, tag="bq4_sb")
            nc.sync.dma_start(bq4[:], bq4_d[:, :])
            c2 = cpool.tile([EPC, S], MMDT, tag="c2_sb")
            nc.sync.dma_start(c2[:], c2_d[:, :])
            psw = cpool.tile([EPC, EPC], MMDT, tag="psw_sb")
            nc.sync.dma_start(psw[:], psw_d[:, :])
            load_xt(0, 1)
            wv = _ldw(wv_d)
            bvb = cpool.tile([EPC, EPC], F32, tag="bvb_sb")
            nc.sync.dma_start(bvb[:], bvb_d[:, :])
            load_xt(0, 2)
            load_xt(0, 3)
            wo = cpool.tile([EPC, HIDDEN], MMDT, tag="wo_sb")
            nc.sync.dma_start(wo[:], wo_d[:, :])
            sel2 = cpool.tile([128, EPC], MMDT, tag="sel2_sb")
            nc.sync.dma_start(sel2[:], sel2_d[:, :])
            # persistent rowsum staging tile; zeroed once so the unused
            # partitions contribute 0 (not garbage) to the selector matmul
            r2 = cpool.tile([128, TQ], MMDT, tag="r2_sb")
            nc.vector.memset(r2[:], 0.0)

            # ---------------- per-batch emission helpers ----------------
            qkv = {}  # b -> (q_sb, k_sb, v_sb)

            def _ensure_tiles(b):
                if b not in qkv:
                    q_sb = qkpool.tile([EPC, S], MMDT, tag="q")
                    # k is stored zero-padded per head (kz0 = [k_h0; 0],
                    # kz1 = [0; k_h1]) so the score matmuls contract over
                    # the full 128 partitions: the zero rows kill the
                    # cross-head terms and the PE never switches into
                    # 64-row tile mode (each mode transition costs ~200ns
                    # of LDW serialization against in-flight matmuls).
                    kz0 = qkpool.tile([EPC, S], MMDT, tag="k0")
                    kz1 = qkpool.tile([EPC, S], MMDT, tag="k1")
                    # zero the dead halves on the idle GpSimd engine so the
                    # DVE rope pipeline isn't delayed behind 2x2us memsets
                    nc.gpsimd.memset(kz0[D:EPC, :], 0.0)
                    nc.gpsimd.memset(kz1[0:D, :], 0.0)
                    v_sb = vpool.tile([128, NTK, 2 * D + 2], MMDT, tag="v")
                    nc.vector.memset(v_sb[:, :, D : D + 1], 1.0)
                    nc.vector.memset(v_sb[:, :, 2 * D + 1 : 2 * D + 2], 1.0)
                    qkv[b] = (q_sb, kz0, kz1, v_sb)
                return qkv[b]

            def _rope_tail(pa, bi, dsts, ts_):
                """RoPE epilogue for one projection chain result pa.
                dsts: list of (tile, row_lo, row_hi) the result is split to."""
                praw = wpool.tile([EPC, TQ], MMDT, tag="praw")
                nc.vector.tensor_copy(praw[:], pa[:])
                t1 = wpool.tile([EPC, TQ], F32, tag="rope")
                nc.vector.scalar_tensor_tensor(
                    t1[:], pa[:], bq4[:, bi : bi + 1], c1[:, ts_],
                    ALU.add, ALU.mult,
                )
                pb = psP.tile([128, TQ], F32, tag="pj")
                nc.tensor.matmul(pb[:], psw[:], praw[:])
                t2 = wpool.tile([EPC, TQ], F32, tag="rope")
                nc.vector.scalar_tensor_tensor(
                    t2[:], pb[:], bq4[:, bi + 1 : bi + 2],
                    c2[:, ts_], ALU.add, ALU.mult,
                )
                for dt, lo, hi in dsts:
                    nc.vector.tensor_add(dt[lo:hi, ts_], t1[lo:hi, :], t2[lo:hi, :])

            def emit_qk_chunk(b, t4, dense=False):
                q_sb, kz0, kz1, v_sb = _ensure_tiles(b)
                if (b, t4) not in xt_tiles:
                    load_xt(b, t4)
                xt_t = xt_tiles[(b, t4)]
                ts_ = slice(t4 * TQ, (t4 + 1) * TQ)
                qdst = [(q_sb, 0, EPC)]
                kdst = [(kz0, 0, D), (kz1, D, EPC)]
                # q/k projections in interleaved head layout (rope pairs are
                # adjacent rows; the dot product is invariant to the order):
                # q' = (q + b) * C1 + (swap(q) + swap(b)) * C2
                if dense:
                    # head variant: run both 8-MM chains back to back (k's
                    # accumulator borrows the out-proj PSUM bank, idle here)
                    # so the PE has no DVE round-trip between them.
                    pa_q = psP.tile([128, TQ], F32, tag="pj")
                    pa_k = psY.tile([128, TQ], F32, tag="y")
                    for wa, pa in ((wq, pa_q), (wk, pa_k)):
                        for dc in range(DC):
                            nc.tensor.matmul(
                                pa[:], wa[:, dc], xt_t[:, dc],
                                start=(dc == 0), stop=(dc == DC - 1),
                            )
                    _rope_tail(pa_q, 0, qdst, ts_)
                    _rope_tail(pa_k, 2, kdst, ts_)
                    return
                for wa, bi, dsts in ((wq, 0, qdst), (wk, 2, kdst)):
                    pa = psP.tile([128, TQ], F32, tag="pj")
                    for dc in range(DC):
                        nc.tensor.matmul(
                            pa[:], wa[:, dc], xt_t[:, dc],
                            start=(dc == 0), stop=(dc == DC - 1),
                        )
                    _rope_tail(pa, bi, dsts, ts_)

            def emit_v_quarter(b, t4, i):
                """v projection for 128 tokens (quarter i of chunk t4)."""
                q_sb, kz0, kz1, v_sb = _ensure_tiles(b)
                xt_t = xt_tiles[(b, t4)]
                vp = psP.tile([128, TK], F32, tag="pj")
                cs = slice(i * TK, (i + 1) * TK)
                for dc in range(DC):
                    nc.tensor.matmul(
                        vp[:], xt_t[:, dc, cs], wv[:, dc],
                        start=(dc == 0), stop=(dc == DC - 1),
                    )
                blk = t4 * (TQ // TK) + i
                # one strided add covers both heads' 64-col halves (the dst
                # skips the ones column at offset D within each 65-col bank)
                dst3 = v_sb[:, blk, 0 : 2 * D + 2].rearrange(
                    "p (s c) -> p s c", c=D + 1
                )[:, :, 0:D]
                nc.vector.tensor_add(
                    dst3,
                    vp[:].rearrange("p (s c) -> p s c", c=D),
                    bvb[:, :].rearrange("p (s c) -> p s c", c=D),
                )

            def emit_norm_a(o0_, o1_):
                """Stage the two rowsum rows for the selector broadcast."""
                nc.vector.tensor_copy(r2[0:1, :], o0_[D : D + 1, :])
                nc.vector.tensor_copy(r2[32:33, :], o1_[D : D + 1, :])

            def emit_norm_b(o0_, o1_):
                """Broadcast rowsums, reciprocal, normalize -> o_sb."""
                bp = psY.tile([128, TQ], F32, tag="y")
                nc.tensor.matmul(bp[:], sel2[:], r2[:])
                rec = wpool.tile([128, TQ], F32, tag="rec")
                nc.vector.reciprocal_approx_fast(rec[:], bp[:])
                o_sb = wpool.tile([128, TQ], MMDT, tag="osb")
                nc.vector.tensor_tensor(
                    o_sb[0:D, :], o0_[0:D, :], rec[0:D, :], ALU.mult
                )
                nc.vector.tensor_tensor(
                    o_sb[D:EPC, :], o1_[0:D, :], rec[D:EPC, :], ALU.mult
                )
                return o_sb

            def emit_outproj_piece(b, tq_, o_sb_, eb):
                yp = psY.tile([128, TQ], F32, tag="y")
                nc.tensor.matmul(
                    yp[:], wo[:, eb * 128 : (eb + 1) * 128], o_sb_[:]
                )
                y_sb = ypool.tile([128, TQ], F32, tag="ysb")
                nc.vector.tensor_copy(y_sb[:], yp[:])
                nc.sync.dma_start(
                    yt_d[
                        eb * 128 : (eb + 1) * 128,
                        b * S + tq_ * TQ : b * S + (tq_ + 1) * TQ,
                    ],
                    y_sb[:],
                )

            # ---------------- pipelined schedule ----------------
            # Chunk sequence: 16 (b, tq) chunks.  PV trails exp by 5 blocks
            # and the last 5 PVs of chunk t DRAIN inside chunk t+1 (tk 0-2),
            # so at every chunk boundary the next chunk's scores issue
            # first and the exp stream never starves.  Chunk t+1 also hosts
            # chunk t's norm (tk 2-3) and out-proj pieces (tk 4..11), plus
            # the projection of a later chunk: qk chain at tk 3, v quarters
            # at tk 6/8/10/12, with the xt DMA prefetched one chunk earlier.
            # proj(b+1, t4) is hosted at (b,1),(b,2),(b,3) and (b+1,0); the
            # last is safe because scores(b+1, 0, tk>=12) only need
            # k(b+1, 3) by block 12 and PV trails five blocks further.
            # Batch 0's own projection is pipelined into chunk (0,0).

            TRAIL = 5  # PV blocks trailing the exp stream

            def emit_dummy(n=2, pool=None, tag="o0"):
                # HAM keep-warm filler for the projection-heavy first chunk:
                # no-reader matmuls into a currently-unused PSUM bank fill
                # PE stall windows so the clock gate stays at 2.4 GHz.
                dum = (pool or psO).tile([128, TQ], F32, tag=tag, name="dum")
                for _ in range(n):
                    nc.tensor.matmul(dum[:, 0:TK], wz[:, 0:TK], wz[:, 0:TK])

            emit_qk_chunk(0, 0, dense=True)
            emit_dummy(2)

            host_qk = {}  # chunk -> (b, t4) projection chunk it hosts
            for b in range(B - 1):
                for t4 in range(NTQ):
                    ci = (b, t4 + 1) if t4 + 1 < NTQ else (b + 1, 0)
                    host_qk[ci] = (b + 1, t4)
            chunks = [(b, tq) for b in range(B) for tq in range(NTQ)]
            prefetch = {}  # chunk -> [(b, t4)] xt tiles to DMA-prefetch
            for ci_idx, ck in enumerate(chunks):
                if ck in host_qk and ci_idx > 0:
                    prefetch.setdefault(chunks[ci_idx - 1], []).append(host_qk[ck])

            # previous chunk's leftovers: (b, tq, o0, o1, [(e_sb, pk, last)])
            prev = None
            for ci, (b, tq) in enumerate(chunks):
                q_sb, kz0, kz1, v_sb = _ensure_tiles(b)
                qs = slice(tq * TQ, (tq + 1) * TQ)
                ot = []  # [o0, o1] allocated lazily at first PV

                slots = {}

                def add_slot(tk, fn):
                    slots.setdefault(tk, []).append(fn)

                if (b, tq) in host_qk:
                    pb, pt4 = host_qk[(b, tq)]
                    add_slot(3, lambda pb=pb, pt4=pt4: emit_qk_chunk(pb, pt4))
                    for j, tkslot in enumerate((6, 8, 10, 12)):
                        add_slot(
                            tkslot,
                            lambda pb=pb, pt4=pt4, j=j: emit_v_quarter(pb, pt4, j),
                        )
                for pb, pt4 in prefetch.get((b, tq), []):
                    add_slot(8, lambda pb=pb, pt4=pt4: load_xt(pb, pt4))
                if ci == 0:
                    # pipeline batch 0's own projection into this chunk:
                    # qk(0,c) must precede scores(tk=4c); v quarter for
                    # kv-block p must precede PV pk=p (trails exp by TRAIL).
                    add_slot(0, lambda: emit_v_quarter(0, 0, 0))
                    add_slot(0, lambda: emit_qk_chunk(0, 1, dense=True))
                    add_slot(1, lambda: emit_v_quarter(0, 0, 1))
                    add_slot(1, lambda: emit_dummy(2))
                    add_slot(2, lambda: emit_v_quarter(0, 0, 2))
                    add_slot(2, lambda: emit_dummy(2))
                    add_slot(3, lambda: emit_qk_chunk(0, 2))
                    add_slot(3, lambda: emit_dummy(2))
                    add_slot(4, lambda: emit_v_quarter(0, 0, 3))
                    add_slot(4, lambda: emit_dummy(2))
                    # later chunk-0 stall windows: fill via the out-proj
                    # bank (free here; its rope readers are long done)
                    add_slot(6, lambda: emit_dummy(2, psY, "y"))
                    add_slot(8, lambda: emit_dummy(2, psY, "y"))
                    add_slot(10, lambda: emit_dummy(2, psY, "y"))
                    add_slot(5, lambda: emit_v_quarter(0, 1, 0))
                    add_slot(6, lambda: emit_qk_chunk(0, 3))
                    add_slot(6, lambda: emit_v_quarter(0, 1, 1))
                    add_slot(7, lambda: emit_v_quarter(0, 1, 2))
                    add_slot(8, lambda: emit_v_quarter(0, 1, 3))
                    add_slot(9, lambda: emit_v_quarter(0, 2, 0))
                    add_slot(9, lambda: emit_v_quarter(0, 2, 1))
                    add_slot(10, lambda: emit_v_quarter(0, 2, 2))
                    add_slot(10, lambda: emit_v_quarter(0, 2, 3))
                    add_slot(11, lambda: emit_v_quarter(0, 3, 0))
                    add_slot(11, lambda: emit_v_quarter(0, 3, 1))
                    add_slot(12, lambda: emit_v_quarter(0, 3, 2))
                    add_slot(12, lambda: emit_v_quarter(0, 3, 3))

                # deferred drain + norm + out-proj of the previous chunk
                if prev is not None:
                    pb_, ptq_, po0, po1, carry = prev
                    pvb = qkv[pb_][3]  # v_sb of the previous chunk's batch

                    def drain2(n, pvb=pvb, po0=po0, po1=po1, carry=carry):
                        for _ in range(n):
                            if not carry:
                                return
                            ce, cpk, clast = carry.pop(0)
                            st = dict(start=False, stop=clast)
                            nc.tensor.matmul(
                                po0[0 : D + 1, :], pvb[:, cpk, 0 : D + 1],
                                ce[:, 0:TQ], **st,
                            )
                            nc.tensor.matmul(
                                po1[0 : D + 1, :], pvb[:, cpk, D + 1 : 2 * D + 2],
                                ce[:, TQ : 2 * TQ], **st,
                            )

                    add_slot(0, lambda d=drain2: d(2))
                    add_slot(1, lambda d=drain2: d(2))
                    add_slot(2, lambda d=drain2: d(1))
                    add_slot(2, lambda po0=po0, po1=po1: emit_norm_a(po0, po1))
                    osb_box = []
                    add_slot(
                        3,
                        lambda po0=po0, po1=po1, bx=osb_box: bx.append(
                            emit_norm_b(po0, po1)
                        ),
                    )
                    for eb in range(HIDDEN // 128):
                        add_slot(
                            4 + eb,
                            lambda pb_=pb_, ptq_=ptq_, bx=osb_box, eb=eb: (
                                emit_outproj_piece(pb_, ptq_, bx[0], eb)
                            ),
                        )
                    prev = None

                def emit_pv(pe_, pk_, stop):
                    if not ot:
                        ot.append(psO.tile([128, TQ], F32, tag="o0", name="o0"))
                        ot.append(psO.tile([128, TQ], F32, tag="o1", name="o1"))
                    st = dict(start=(pk_ == 0), stop=stop)
                    nc.tensor.matmul(
                        ot[0][0 : D + 1, :], v_sb[:, pk_, 0 : D + 1],
                        pe_[:, 0:TQ], **st,
                    )
                    nc.tensor.matmul(
                        ot[1][0 : D + 1, :], v_sb[:, pk_, D + 1 : 2 * D + 2],
                        pe_[:, TQ : 2 * TQ], **st,
                    )

                last = ci + 1 == len(chunks)
                pend = []
                sps = {}
                for tk in range(NTK):
                    if tk % 2 == 0:
                        # emit two blocks' score matmuls together (keeps the
                        # exp stream two blocks deep at chunk boundaries)
                        for t2 in (tk, tk + 1):
                            ks = slice(t2 * TK, (t2 + 1) * TK)
                            sp = psA.tile([128, 2 * TQ], F32, tag="big", name="sp")
                            nc.tensor.matmul(
                                sp[:, 0:TQ], kz0[:, ks], q_sb[:, qs]
                            )
                            nc.tensor.matmul(
                                sp[:, TQ : 2 * TQ], kz1[:, ks], q_sb[:, qs]
                            )
                            sps[t2] = sp
                    e_sb = epool.tile([128, 2 * TQ], MMDT, tag="e")
                    nc.scalar.activation(
                        e_sb[:], sps.pop(tk), mybir.ActivationFunctionType.Exp
                    )
                    for fn in slots.get(tk, ()):
                        fn()
                    pend.append((e_sb, tk))
                    npop = 1 if len(pend) > TRAIL else 0
                    if last and tk >= 8 and len(pend) > 2:
                        # catch the PV stream up during the final chunk so
                        # the tail after the last exp is short
                        npop = max(npop, 2 if len(pend) > 3 else 1)
                    for _ in range(npop):
                        pe_, pk_ = pend.pop(0)
                        emit_pv(pe_, pk_, False)
                if ci + 1 < len(chunks):
                    # leave the trailing PVs to drain inside the next chunk
                    carry = [
                        (pe_, pk_, pk_ == NTK - 1) for pe_, pk_ in pend
                    ]
                    prev = (b, tq, ot[0], ot[1], carry)
                else:
                    while pend:
                        pe_, pk_ = pend.pop(0)
                        emit_pv(pe_, pk_, pk_ == NTK - 1)
                    prev = (b, tq, ot[0], ot[1], [])

            # final chunk: norm + out-proj immediately; alternate the
            # out-proj accumulator between the two spare PSUM banks so the
            # tail isn't serialized on a single buffer's copy-back.
            pb_, ptq_, po0, po1, _ = prev
            emit_norm_a(po0, po1)
            posb = emit_norm_b(po0, po1)
            for eb in range(HIDDEN // 128):
                pool = psY if eb % 2 == 0 else psP
                yp = pool.tile([128, TQ], F32, tag="y" if eb % 2 == 0 else "pj")
                nc.tensor.matmul(
                    yp[:], wo[:, eb * 128 : (eb + 1) * 128], posb[:]
                )
                y_sb = ypool.tile([128, TQ], F32, tag="ysb")
                # alternate the copy-back between DVE and the now-idle ACT
                # engine so the serial tail halves
                if eb % 2 == 0:
                    nc.vector.tensor_copy(y_sb[:], yp[:])
                else:
                    nc.scalar.copy(y_sb[:], yp[:])
                nc.sync.dma_start(
                    yt_d[
                        eb * 128 : (eb + 1) * 128,
                        pb_ * S + ptq_ * TQ : pb_ * S + (ptq_ + 1) * TQ,
                    ],
                    y_sb[:],
                )

    nc.compile()
    return nc


def host_prep(x, Wq, bq, Wk, bk, Wv, bv, Wo, bo):
    """Build the 8 per-core input maps from the full-size inputs."""
    B, S, _ = x.shape
    T = B * S
    xt = to_mm(x.reshape(T, HIDDEN).T)

    # rope tables in INTERLEAVED head layout: row 2j and 2j+1 share
    # frequency j.  q'[2j] = q[2j] c_j - q[2j+1] s_j ;
    # q'[2j+1] = q[2j] s_j + q[2j+1] c_j.  With swap() exchanging rows
    # 2j <-> 2j+1:  q' = q * C1 + swap(q) * C2,
    # C1[2j]=C1[2j+1]=c_j, C2[2j]=-s_j, C2[2j+1]=+s_j.
    j = np.arange(D // 2)
    inv_freq = 1.0 / (ROPE_BASE ** (2 * j / D))
    t = np.arange(S, dtype=np.float64)
    fr = np.outer(t, inv_freq)  # [S, 32]
    cf = np.cos(fr).T  # [32, S]
    sf = np.sin(fr).T
    c1h = np.repeat(cf, 2, axis=0)  # [64, S]
    c2h = np.empty((D, S))
    c2h[0::2] = -sf
    c2h[1::2] = sf
    c1 = np.ascontiguousarray(np.tile(c1h, (HPC, 1)).astype(np.float32))
    c2 = np.ascontiguousarray(np.tile(c2h, (HPC, 1)).astype(np.float32))

    # adjacent-pair swap permutation (within the 128 local rows)
    swp = np.arange(EPC)
    swp = swp ^ 1  # 2j <-> 2j+1
    psw = np.zeros((EPC, EPC), np.float32)
    psw[swp, np.arange(EPC)] = 1.0

    sel2 = np.zeros((128, EPC), np.float32)
    sel2[0, 0:D] = 1.0
    sel2[32, D : 2 * D] = 1.0

    scale = 1.0 / np.sqrt(D)
    in_maps = []
    for c in range(NCORES):
        rows = slice(c * EPC, (c + 1) * EPC)
        Rq, Rk, Rv = Wq[rows], Wk[rows], Wv[rows]
        bqc, bkc, bvc = bq[rows], bk[rows], bv[rows]
        m = {
            "xt": xt,
            "wq": to_mm((Rq * scale).T),
            "wk": to_mm(Rk.T),
            "wv": to_mm(Rv.T),
            "wo": to_mm(Wo[:, rows].T),
            "psw": to_mm(psw),
            "bq4": np.ascontiguousarray(
                np.stack(
                    [bqc * scale, bqc[swp] * scale, bkc, bkc[swp]], 1
                ).astype(np.float32)
            ),
            "bvb": np.ascontiguousarray(
                np.tile(bvc[None, :], (EPC, 1)).astype(np.float32)
            ),
            "c1": to_mm(c1),
            "c2": to_mm(c2),
            "sel2": to_mm(sel2),
        }
        in_maps.append(m)
    return in_maps


_NC_CACHE = {}


def _get_nc(B, S):
    key = (B, S)
    if key not in _NC_CACHE:
        _NC_CACHE[key] = build_nc(B, S)
    return _NC_CACHE[key]


def run_cores(in_maps, B, S, trace=False):
    nc = _get_nc(B, S)
    return run_bass_kernel_spmd(
        nc, in_maps, core_ids=list(range(NCORES)), trace=trace
    )


def gather(results, bo, B, S):
    acc = results[0]["yt"].astype(np.float32)
    for c in range(1, NCORES):
        acc = acc + results[c]["yt"]
    y = acc.T + bo[None, :]
    return np.ascontiguousarray(y.reshape(B, S, HIDDEN).astype(np.float32))


def kernel(x, Wq, bq, Wk, bk, Wv, bv, Wo, bo):
    x = np.asarray(x, np.float32)
    B, S, _ = x.shape
    in_maps = host_prep(
        x,
        np.asarray(Wq, np.float32), np.asarray(bq, np.float32),
        np.asarray(Wk, np.float32), np.asarray(bk, np.float32),
        np.asarray(Wv, np.float32), np.asarray(bv, np.float32),
        np.asarray(Wo, np.float32), np.asarray(bo, np.float32),
    )
    res = run_cores(in_maps, B, S, trace=False)
    return gather(res.results, np.asarray(bo, np.float32), B, S)
